# revision 1
# baseline (speedup 1.0000x reference)
"""2-layer GCN (GCNConv -> ReLU -> GCNConv -> log_softmax) on 8 TRN2 NeuronCores.

Strategy (dest-sharded, per the halo-exchange hint):
  - Nodes (and their incident edges, by destination) are partitioned across the
    8 cores: core c owns destination nodes [c*12500, (c+1)*12500).
  - gcn_norm coefficients (degrees / D^-1/2 scaling) are host-side graph
    preprocessing; the per-edge coefficient w' = dinv[src]*w*dinv[dst] is
    folded into one scalar per edge.
  - The halo exchange (gather of remote source features) is materialized on
    the host as a per-core, destination-ordered ELLPACK message stream: this
    turns the device-side work into pure sequential streaming.  (Measured on
    this hardware: every device-side random-access primitive - indirect DMA,
    dma_gather, gpsimd ap_gather - costs 25-200ns per edge, which is 10-100x
    slower than streaming; so the gather is done at input-layout time.)
  - Device kernel 1: stream x-messages, weighted segment-sum over edge slots
    (DVE), transpose (PE), @W1 + b1, ReLU  -> z shard per core.
  - Host: second halo exchange - gather z rows into layer-2 message streams.
  - Device kernel 2: stream z-messages, weighted segment-sum, @W2 + b2,
    log_softmax -> output shard per core.
  - Linear layers are applied AFTER aggregation (A@(X@W1) == (A@X)@W1), so
    all feature arithmetic (the actual FLOPs) happens on device.
"""

import sys

sys.path.insert(0, "/opt/trn_rl_repo")

import numpy as np

from concourse import bass, mybir, bacc
import concourse.tile as tile
from concourse import bass_utils
from concourse.masks import make_identity

N = 100_000
NCORES = 8
DPC = N // NCORES            # 12500 dests per core
P = 128                      # partitions
NWIN = (DPC + P - 1) // P    # 98 windows of 128 dests
DPC_PAD = NWIN * P           # 12544

F_IN = 37
H = 16
C = 2


# ----------------------------------------------------------------------------
# Host-side graph preprocessing (indices / weights only - no feature math)
# ----------------------------------------------------------------------------

def preprocess_graph(edge_index, edge_weight):
    row = np.asarray(edge_index[0]).astype(np.int64)
    col = np.asarray(edge_index[1]).astype(np.int64)
    w = np.asarray(edge_weight).astype(np.float32)

    loop = np.arange(N, dtype=np.int64)
    row = np.concatenate([row, loop])
    col = np.concatenate([col, loop])
    w = np.concatenate([w, np.ones(N, np.float32)])

    deg = np.bincount(col, weights=w.astype(np.float64), minlength=N)
    dinv = np.where(deg > 0, 1.0 / np.sqrt(deg), 0.0).astype(np.float32)
    wn = dinv[row] * w * dinv[col]  # [E+N] f32

    core = col // DPC
    shards = []
    for c in range(NCORES):
        m = core == c
        shards.append((row[m], col[m] - c * DPC, wn[m]))

    # per-core degree-sorted dest permutation (uniform geometry across cores)
    perms, counts_sorted = [], []
    for c in range(NCORES):
        _, ld, _ = shards[c]
        cnt = np.bincount(ld, minlength=DPC)
        order = np.argsort(-cnt, kind="stable")       # rank -> local dest
        permpos = np.empty(DPC, np.int64)
        permpos[order] = np.arange(DPC)               # local dest -> rank
        perms.append((order, permpos))
        cs = np.zeros(DPC_PAD, np.int64)
        cs[: DPC] = cnt[order]
        counts_sorted.append(cs)

    # shared window widths: max over cores of max count within each window
    cnt_all = np.stack(counts_sorted)                 # [8, 12544]
    Lw = cnt_all.reshape(NCORES, NWIN, P).max(axis=(0, 2)).astype(np.int64)
    Lw = np.maximum(Lw, 1)
    off = np.concatenate([[0], np.cumsum(Lw)])
    S = int(off[-1])

    # per-core slot assignment: (128, S) arrays of src node id and w'
    srcpos_all, wn_all = [], []
    for c in range(NCORES):
        src, ld, wnc = shards[c]
        _, permpos = perms[c]
        q = permpos[ld]                                # rank of each edge's dest
        sort = np.argsort(q, kind="stable")
        qs, srcs, wns = q[sort], src[sort], wnc[sort]
        # within-dest slot index
        cnt = np.bincount(qs, minlength=DPC_PAD)
        starts = np.concatenate([[0], np.cumsum(cnt)])[:-1]
        slot = np.arange(len(qs)) - starts[qs]
        wi = qs // P
        colidx = off[wi] + slot
        pi = qs % P
        sp = np.zeros((P, S), np.int64)
        wa = np.zeros((P, S), np.float32)
        sp[pi, colidx] = srcs
        wa[pi, colidx] = wns
        srcpos_all.append(sp)
        wn_all.append(wa)

    return {
        "Lw": Lw, "off": off, "S": S,
        "srcpos": srcpos_all, "wn": wn_all, "perms": perms,
    }


# ----------------------------------------------------------------------------
# Device program: stream messages -> weighted segment-sum -> @W + b -> act
# ----------------------------------------------------------------------------

def build_layer_program(F, OutF, S, Lw, off, last, loop_reps=1):
    """F: message width (37 or 16). OutF: output width (16 or 2).
    last: if True apply log_softmax epilogue, else ReLU."""
    nc = bacc.Bacc("TRN2", target_bir_lowering=False, debug=False,
                   num_devices=NCORES)
    f32 = mybir.dt.float32
    msg_d = nc.dram_tensor("msg", [P, S * F], f32, kind="ExternalInput").ap()
    wn_d = nc.dram_tensor("wn", [P, S], f32, kind="ExternalInput").ap()
    W_d = nc.dram_tensor("W", [F, OutF], f32, kind="ExternalInput").ap()
    b_d = nc.dram_tensor("b", [P, OutF], f32, kind="ExternalInput").ap()
    out_d = nc.dram_tensor("out", [DPC_PAD, OutF], f32, kind="ExternalOutput").ap()
    out_v = out_d.rearrange("(w p) f -> p w f", p=P)

    BATCH = 8  # windows per staged output DMA
    maxL = int(max(Lw))

    with tile.TileContext(nc) as tc:
        with tc.tile_pool(name="const", bufs=1) as cpool, \
             tc.tile_pool(name="sbuf", bufs=3) as pool, \
             tc.tile_pool(name="psum", bufs=2, space="PSUM") as ppool:
            wn_sb = cpool.tile([P, S], f32)
            W_sb = cpool.tile([F, OutF], f32)
            b_sb = cpool.tile([P, OutF], f32)
            ident = cpool.tile([P, P], f32)
            nc.sync.dma_start(out=wn_sb[:], in_=wn_d[:])
            nc.sync.dma_start(out=W_sb[:], in_=W_d[:])
            nc.sync.dma_start(out=b_sb[:], in_=b_d[:])
            make_identity(nc, ident[:])

            def windows():
                stage = None
                for w in range(NWIN):
                    L, o = int(Lw[w]), int(off[w])
                    if w % BATCH == 0:
                        stage = pool.tile([P, BATCH * OutF], f32, tag="stage")
                    msg = pool.tile([P, maxL * F], f32, tag="msg")
                    nc.sync.dma_start(out=msg[:, : L * F],
                                      in_=msg_d[:, o * F:(o + L) * F])
                    m3 = msg[:, : L * F].rearrange("p (s f) -> p s f", f=F)
                    wb = wn_sb[:, o:o + L].unsqueeze(-1).to_broadcast([P, L, F])
                    nc.vector.tensor_tensor(out=m3, in0=m3, in1=wb,
                                            op=mybir.AluOpType.mult)
                    agg = pool.tile([P, F], f32, tag="agg")
                    mr = msg[:, : L * F].rearrange("p (s f) -> p f s", f=F)
                    nc.vector.tensor_reduce(out=agg[:], in_=mr,
                                            axis=mybir.AxisListType.X,
                                            op=mybir.AluOpType.add)
                    # transpose agg [128,F] -> [F,128], then @W -> [128,OutF]
                    aggT_p = ppool.tile([F, P], f32, tag="aggT_p")
                    nc.tensor.transpose(out=aggT_p[:], in_=agg[:], identity=ident[:])
                    aggT = pool.tile([F, P], f32, tag="aggT")
                    nc.scalar.copy(out=aggT[:], in_=aggT_p[:])
                    h_p = ppool.tile([P, OutF], f32, tag="h_p")
                    nc.tensor.matmul(out=h_p[:], lhsT=aggT[:], rhs=W_sb[:],
                                     start=True, stop=True)
                    sl = stage[:, (w % BATCH) * OutF:(w % BATCH + 1) * OutF]
                    if not last:
                        zt = pool.tile([P, OutF], f32, tag="zt")
                        nc.vector.tensor_tensor(out=zt[:], in0=h_p[:], in1=b_sb[:],
                                                op=mybir.AluOpType.add)
                        nc.scalar.activation(out=sl, in_=zt[:],
                                             func=mybir.ActivationFunctionType.Relu)
                    else:
                        ot = pool.tile([P, OutF], f32, tag="ot")
                        nc.vector.tensor_tensor(out=ot[:], in0=h_p[:], in1=b_sb[:],
                                                op=mybir.AluOpType.add)
                        rmax = pool.tile([P, 1], f32, tag="rmax")
                        nc.vector.tensor_reduce(out=rmax[:], in_=ot[:],
                                                axis=mybir.AxisListType.X,
                                                op=mybir.AluOpType.max)
                        xm = pool.tile([P, OutF], f32, tag="xm")
                        nc.vector.tensor_scalar_sub(xm[:], ot[:], rmax[:])
                        ex = pool.tile([P, OutF], f32, tag="ex")
                        se = pool.tile([P, 1], f32, tag="se")
                        nc.scalar.activation(out=ex[:], in_=xm[:],
                                             func=mybir.ActivationFunctionType.Exp,
                                             accum_out=se[:])
                        lse = pool.tile([P, 1], f32, tag="lse")
                        nc.scalar.activation(out=lse[:], in_=se[:],
                                             func=mybir.ActivationFunctionType.Ln)
                        nc.vector.tensor_scalar_sub(sl, xm[:], lse[:])
                    if w % BATCH == BATCH - 1 or w == NWIN - 1:
                        w0 = (w // BATCH) * BATCH
                        nwin = w - w0 + 1
                        nc.scalar.dma_start(
                            out=out_v[:, w0:w0 + nwin, :],
                            in_=stage[:, : nwin * OutF].rearrange(
                                "p (w f) -> p w f", f=OutF))

            if loop_reps == 1:
                windows()
            else:
                with tc.For_i(0, loop_reps, 1):
                    windows()
    nc.compile()
    return nc


# ----------------------------------------------------------------------------
# Full model
# ----------------------------------------------------------------------------

_CACHE = {}


def _get_programs(S, Lw, off, loop_reps=1):
    key = (S, tuple(Lw), loop_reps)
    if key not in _CACHE:
        k1 = build_layer_program(F_IN, H, S, Lw, off, last=False,
                                 loop_reps=loop_reps)
        k2 = build_layer_program(H, C, S, Lw, off, last=True,
                                 loop_reps=loop_reps)
        _CACHE[key] = (k1, k2)
    return _CACHE[key]


def kernel(x, edge_index, edge_weight, W1, b1, W2, b2, _loop_reps=1,
           _return_all=False):
    x = np.asarray(x, dtype=np.float32)
    W1 = np.asarray(W1, np.float32); b1 = np.asarray(b1, np.float32)
    W2 = np.asarray(W2, np.float32); b2 = np.asarray(b2, np.float32)

    g = preprocess_graph(edge_index, edge_weight)
    S, Lw, off = g["S"], g["Lw"], g["off"]
    k1, k2 = _get_programs(S, Lw, off, _loop_reps)

    b1r = np.broadcast_to(b1, (P, H)).copy()
    in1 = []
    for c in range(NCORES):
        msgx = x[g["srcpos"][c].ravel()].reshape(P, S * F_IN)
        in1.append({"msg": msgx, "wn": g["wn"][c], "W": W1, "b": b1r})
    r1 = bass_utils.run_bass_kernel_spmd(k1, in1, core_ids=list(range(NCORES)))
    zshards = [r1.results[c]["out"] for c in range(NCORES)]  # [12544, 16] each

    # host halo exchange for layer 2: map node id -> row in stacked z shards
    posmap = np.empty(N, np.int64)
    for c in range(NCORES):
        _, permpos = g["perms"][c]
        posmap[c * DPC:(c + 1) * DPC] = c * DPC_PAD + permpos
    zfull = np.concatenate(zshards, axis=0)  # [8*12544, 16]

    b2r = np.broadcast_to(b2, (P, C)).copy()
    in2 = []
    for c in range(NCORES):
        msgz = zfull[posmap[g["srcpos"][c].ravel()]].reshape(P, S * H)
        in2.append({"msg": msgz, "wn": g["wn"][c], "W": W2, "b": b2r})
    r2 = bass_utils.run_bass_kernel_spmd(k2, in2, core_ids=list(range(NCORES)))

    out = np.empty((N, C), np.float32)
    for c in range(NCORES):
        order, _ = g["perms"][c]
        shard = r2.results[c]["out"]          # [12544, C], row q = rank q
        out[c * DPC + order] = shard[: DPC]
    if _return_all:
        return out, zshards, g
    return out



# revision 5
# speedup vs baseline: 15.0706x; 15.0706x over previous
"""2-layer GCN (GCNConv -> ReLU -> GCNConv -> log_softmax) on 8 TRN2 NeuronCores.

v2 design (dest-sharded halo-exchange, transform-before-gather):

  - Nodes (and their incident edges, by destination) are partitioned across
    the 8 cores: core c owns destinations [c*12500, (c+1)*12500).
  - gcn_norm coefficients are host-side graph preprocessing; per-edge
    w' = dinv[src]*w*dinv[dst] is one fp16 scalar per edge slot.
  - The dense transforms run on device BEFORE each gather (exactly the
    reference's own order: h = x @ W first), so the halo-exchange message
    streams carry 16-wide (layer 1) and 2-wide (layer 2) features instead
    of 37-wide raw x:
      kernel A: h = x @ W1 on device (PE)           -> host gathers h-messages
      kernel B: weighted segment-sum of h-messages (custom fused DVE
                scan op), z = relu(agg + b1), zw = z @ W2 on device (PE)
                                                     -> host gathers zw-messages
      kernel C: weighted segment-sum of zw-messages, + b2, 2-class
                log_softmax (softplus form) on device.
  - The weighted segment-sum is ONE fused DVE pass: a custom DVE op
    computes the running prefix-sum of msg*wn; per-destination sums are
    prefix differences, extracted with stride-L access patterns (windows
    are degree-sorted and padded to a shared per-batch width L so the
    extraction is a single strided instruction per batch).
  - Messages are fp16 (halves HBM traffic; accumulation is fp32 on-chip).
"""

import sys

sys.path.insert(0, "/opt/trn_rl_repo")

import numpy as np

from concourse import bass, mybir, bacc
import concourse.tile as tile
from concourse import bass_utils
from concourse.masks import make_identity

N = 100_000
NCORES = 8
DPC = N // NCORES            # 12500 dests per core
P = 128                      # partitions
NWIN = (DPC + P - 1) // P    # 98 windows of 128 dests
DPC_PAD = NWIN * P           # 12544
WB = 14                      # windows per batch (shared slot width L)
NB = NWIN // WB              # 7 batches
NPAD = 12800                 # 25 * 512, padded per-core node count (kernel A)

F_IN = 37
H = 16
C = 2

SCAN_MUL = None
ADD_RELU = None


def register_custom_ops():
    """Register the fused DVE ops (documented extension API: dve_ops.OPS)."""
    global SCAN_MUL, ADD_RELU
    if SCAN_MUL is not None:
        return
    from concourse import dve_ops as DO
    from concourse.dve_spec import (Spec, Src0, Src1, Zero, AluOp, scan,
                                    maxx, lower, _has_src1)
    from concourse.dve_uop import DveOpSpec
    from concourse.dve_table_gen import dve_ver_for

    def _register(name, spec, subdim=False):
        by_name = {op.name: op for op in DO.OPS}
        if name in by_name:
            return by_name[name]
        op = DO.DveOp(name=name, spec=spec, subdim=subdim, uops_sha={})
        DO.OPS.append(op)
        DO.CUSTOM_DVE_SPECS[name] = spec
        DO._SUB_OPCODE_FOR_NAME[name] = DO._CUSTOM_DVE_ROW_BASE + len(DO.OPS) - 1
        ver = dve_ver_for("TRN2")
        uops = lower(spec, ver=ver)
        tmp = DveOpSpec(name=name, opcode=DO.get_dve_sub_opcode(name),
                        uops=uops, rd1_en=_has_src1(spec))
        object.__setattr__(op, "uops_sha", {ver: tmp.sha(ver)})
        return op

    SCAN_MUL = _register(
        "SCAN_MUL_ANT",
        Spec(body=scan(AluOp.ADD, Src0 * Src1),
             reference=lambda in0, in1: np.cumsum(
                 in0.astype(np.float32) * in1.astype(np.float32), axis=-1)))
    ADD_RELU = _register(
        "ADD_RELU_ANT",
        Spec(body=maxx(Src0 + Src1, Zero),
             reference=lambda in0, in1: np.maximum(in0 + in1, 0)))


# ----------------------------------------------------------------------------
# Host-side graph preprocessing (indices / weights only - no feature math)
# ----------------------------------------------------------------------------

def preprocess_graph(edge_index, edge_weight):
    row = np.asarray(edge_index[0]).astype(np.int64)
    col = np.asarray(edge_index[1]).astype(np.int64)
    w = np.asarray(edge_weight).astype(np.float32)

    loop = np.arange(N, dtype=np.int64)
    row = np.concatenate([row, loop])
    col = np.concatenate([col, loop])
    w = np.concatenate([w, np.ones(N, np.float32)])

    deg = np.bincount(col, weights=w.astype(np.float64), minlength=N)
    dinv = np.where(deg > 0, 1.0 / np.sqrt(deg), 0.0).astype(np.float32)
    wn = dinv[row] * w * dinv[col]  # [E+N] f32

    core = col // DPC
    shards = []
    for c in range(NCORES):
        m = core == c
        shards.append((row[m], col[m] - c * DPC, wn[m]))

    # per-core degree-sorted dest permutation (uniform geometry across cores)
    perms, counts_sorted = [], []
    for c in range(NCORES):
        _, ld, _ = shards[c]
        cnt = np.bincount(ld, minlength=DPC)
        order = np.argsort(-cnt, kind="stable")       # rank -> local dest
        permpos = np.empty(DPC, np.int64)
        permpos[order] = np.arange(DPC)               # local dest -> rank
        perms.append((order, permpos))
        cs = np.zeros(DPC_PAD, np.int64)
        cs[:DPC] = cnt[order]
        counts_sorted.append(cs)

    # shared per-batch slot width: max count within each 14-window batch,
    # over all cores
    cnt_all = np.stack(counts_sorted)                 # [8, 12544]
    Lb = cnt_all.reshape(NCORES, NB, WB * P).max(axis=(0, 2)).astype(np.int64)
    Lb = np.maximum(Lb, 1)
    off = np.concatenate([[0], np.cumsum(WB * Lb)])   # slot offsets per batch
    T = int(off[-1])

    # per-core slot assignment: [P, T] arrays of src node id and fp16 w'
    srcpos_all, wn_all = [], []
    for c in range(NCORES):
        src, ld, wnc = shards[c]
        _, permpos = perms[c]
        q = permpos[ld]                                # rank of each edge's dest
        sort = np.argsort(q, kind="stable")
        qs, srcs, wns = q[sort], src[sort], wnc[sort]
        cnt = np.bincount(qs, minlength=DPC_PAD)
        starts = np.concatenate([[0], np.cumsum(cnt)])[:-1]
        slot = np.arange(len(qs)) - starts[qs]
        wg = qs // P                                   # global window
        b = wg // WB
        t = off[b] + (wg % WB) * Lb[b] + slot
        pi = qs % P
        sp = np.zeros((P, T), np.int64)
        wa = np.zeros((P, T), np.float16)
        sp[pi, t] = srcs
        wa[pi, t] = wns.astype(np.float16)
        srcpos_all.append(sp)
        wn_all.append(wa)

    return {"Lb": Lb, "off": off, "T": T,
            "srcpos": srcpos_all, "wn": wn_all, "perms": perms}


# ----------------------------------------------------------------------------
# Device programs
# ----------------------------------------------------------------------------

def build_xw_program():
    """Kernel A: hT = (x @ W1).T for this core's 12800 (padded) nodes."""
    nc = bacc.Bacc("TRN2", target_bir_lowering=False, debug=False,
                   num_devices=NCORES)
    f32, f16 = mybir.dt.float32, mybir.dt.float16
    xT_d = nc.dram_tensor("xT", [F_IN, NPAD], f16, kind="ExternalInput").ap()
    W1_d = nc.dram_tensor("W1", [F_IN, H], f16, kind="ExternalInput").ap()
    hT_d = nc.dram_tensor("hT", [H, NPAD], f16, kind="ExternalOutput").ap()

    CH = 512
    NCH = NPAD // CH           # 25 chunks
    GRP = 5                    # chunks per staged output DMA
    with tile.TileContext(nc) as tc:
        with tc.tile_pool(name="const", bufs=1) as cpool, \
             tc.tile_pool(name="sbuf", bufs=3) as pool, \
             tc.tile_pool(name="psum", bufs=4, space="PSUM") as ppool:
            xt = cpool.tile([F_IN, NPAD], f16)
            w1 = cpool.tile([F_IN, H], f16)
            nc.sync.dma_start(out=xt[:], in_=xT_d[:])
            nc.sync.dma_start(out=w1[:], in_=W1_d[:])
            stage = None
            for ci in range(NCH):
                if ci % GRP == 0:
                    stage = pool.tile([H, GRP * CH], f16, tag="stage")
                hp = ppool.tile([H, CH], f32, tag="hp")
                nc.tensor.matmul(out=hp[:], lhsT=w1[:],
                                 rhs=xt[:, ci * CH:(ci + 1) * CH],
                                 start=True, stop=True)
                sl = stage[:, (ci % GRP) * CH:(ci % GRP + 1) * CH]
                if ci % 2 == 0:
                    nc.scalar.copy(out=sl, in_=hp[:])
                else:
                    nc.vector.tensor_copy(out=sl, in_=hp[:])
                if ci % GRP == GRP - 1:
                    c0 = (ci // GRP) * GRP * CH
                    nc.scalar.dma_start(out=hT_d[:, c0:c0 + GRP * CH],
                                        in_=stage[:])
    nc.compile()
    return nc


def build_aggB_program(Lb, off, T):
    """Kernel B: fused scan segment-sum of 16-wide h-messages,
    z = relu(agg+b1), zwT = (z @ W2).T   -> zw_d [2, 12544]."""
    register_custom_ops()
    nc = bacc.Bacc("TRN2", target_bir_lowering=False, debug=False,
                   num_devices=NCORES)
    f32, f16 = mybir.dt.float32, mybir.dt.float16
    msg_d = nc.dram_tensor("msg", [P, H * T], f16, kind="ExternalInput").ap()
    wn_d = nc.dram_tensor("wn", [P, T], f16, kind="ExternalInput").ap()
    b1_d = nc.dram_tensor("b1", [P, H], f32, kind="ExternalInput").ap()
    W2_d = nc.dram_tensor("W2", [H, C], f32, kind="ExternalInput").ap()
    zw_d = nc.dram_tensor("zw", [C, DPC_PAD], f32, kind="ExternalOutput").ap()

    S16max = int(H * WB * max(Lb))
    PW = 4                     # windows per psum zw tile (4*128 = 512 fp32)
    with tile.TileContext(nc) as tc:
        with tc.tile_pool(name="const", bufs=1) as cpool, \
             tc.tile_pool(name="msgp", bufs=2) as mpool, \
             tc.tile_pool(name="sbuf", bufs=2) as pool, \
             tc.tile_pool(name="psum", bufs=3, space="PSUM") as ppool:
            wn_sb = cpool.tile([P, T], f16)
            b1_sb = cpool.tile([P, H], f32)
            W2_sb = cpool.tile([H, C], f32)
            ident = cpool.tile([P, P], f32)
            nc.sync.dma_start(out=wn_sb[:], in_=wn_d[:])
            nc.sync.dma_start(out=b1_sb[:], in_=b1_d[:])
            nc.sync.dma_start(out=W2_sb[:], in_=W2_d[:])
            make_identity(nc, ident[:])

            for b in range(NB):
                L = int(Lb[b])
                o = int(off[b])
                WL = WB * L
                S16 = H * WL
                msg = mpool.tile([P, S16max], f16, tag="msg")
                nc.sync.dma_start(out=msg[:, :S16],
                                  in_=msg_d[:, H * o:H * o + S16])
                scf = pool.tile([P, S16max + int(max(Lb)) + 2], f32, tag="scf")
                nc.scalar.memzero(scf[:, 0:1])
                wnb = wn_sb[:, o:o + WL].unsqueeze(1).to_broadcast([P, H, WL])
                nc.vector._custom_dve(SCAN_MUL, out=scf[:, 1:1 + S16],
                                      in0=msg[:, :S16], in1=wnb)
                R = scf[:, :(H * WB + 1) * L].rearrange("p (x s) -> p x s",
                                                        s=L)
                agg = pool.tile([P, H * WB], f32, tag="agg")
                nc.vector.tensor_tensor(
                    out=agg[:], in0=R[:, 1:1 + H * WB, 0:1].squeeze(-1),
                    in1=R[:, 0:H * WB, 0:1].squeeze(-1),
                    op=mybir.AluOpType.subtract)
                z = pool.tile([P, H * WB], f32, tag="z")
                b1b = b1_sb[:].unsqueeze(-1).to_broadcast([P, H, WB])
                nc.vector._custom_dve(
                    ADD_RELU, out=z[:].rearrange("p (f w) -> p f w", w=WB),
                    in0=agg[:].rearrange("p (f w) -> p f w", w=WB), in1=b1b)
                # zw = z @ W2, transposed out, PE+ACT (off the DVE)
                zv = z[:].rearrange("p (f w) -> p f w", w=WB)
                zwT = pool.tile([C, WB * P], f32, tag="zwT")
                zwT_p = None
                for wi in range(WB):
                    if wi % PW == 0:
                        zwT_p = ppool.tile([C, PW * P], f32, tag="zwT_p")
                    zT_p = ppool.tile([H, P], f32, tag="zT_p")
                    nc.tensor.transpose(out=zT_p[:],
                                        in_=zv[:, :, wi:wi + 1].squeeze(-1),
                                        identity=ident[:])
                    zT = pool.tile([H, P], f32, tag="zT")
                    nc.scalar.copy(out=zT[:], in_=zT_p[:])
                    nc.tensor.matmul(out=zwT_p[:, (wi % PW) * P:
                                                 (wi % PW + 1) * P],
                                     lhsT=W2_sb[:], rhs=zT[:],
                                     start=True, stop=True)
                    if wi % PW == PW - 1 or wi == WB - 1:
                        w0 = (wi // PW) * PW
                        nw = wi - w0 + 1
                        nc.scalar.copy(out=zwT[:, w0 * P:(w0 + nw) * P],
                                       in_=zwT_p[:, :nw * P])
                nc.sync.dma_start(out=zw_d[:, b * WB * P:(b + 1) * WB * P],
                                  in_=zwT[:])
    nc.compile()
    return nc


def build_aggC_program(Lb, off, T):
    """Kernel C: fused scan segment-sum of 2-wide zw-messages, + b2,
    2-class log_softmax (softplus form)   -> out [12544, 2]."""
    register_custom_ops()
    nc = bacc.Bacc("TRN2", target_bir_lowering=False, debug=False,
                   num_devices=NCORES)
    f32, f16 = mybir.dt.float32, mybir.dt.float16
    msg_d = nc.dram_tensor("msg", [P, C * T], f16, kind="ExternalInput").ap()
    wn_d = nc.dram_tensor("wn", [P, T], f16, kind="ExternalInput").ap()
    db2_d = nc.dram_tensor("db2", [P, 1], f32, kind="ExternalInput").ap()
    out_d = nc.dram_tensor("out", [DPC_PAD, C], f32,
                           kind="ExternalOutput").ap()
    out_v = out_d.rearrange("(w p) f -> p w f", p=P)

    S2max = int(C * WB * max(Lb))
    with tile.TileContext(nc) as tc:
        with tc.tile_pool(name="const", bufs=1) as cpool, \
             tc.tile_pool(name="sbuf", bufs=2) as pool:
            wn_sb = cpool.tile([P, T], f16)
            db2_sb = cpool.tile([P, 1], f32)
            nc.sync.dma_start(out=wn_sb[:], in_=wn_d[:])
            nc.sync.dma_start(out=db2_sb[:], in_=db2_d[:])

            for b in range(NB):
                L = int(Lb[b])
                o = int(off[b])
                WL = WB * L
                S2 = C * WL
                msg = pool.tile([P, S2max], f16, tag="msg")
                nc.sync.dma_start(out=msg[:, :S2],
                                  in_=msg_d[:, C * o:C * o + S2])
                scf = pool.tile([P, S2max + int(max(Lb)) + 2], f32, tag="scf")
                nc.scalar.memzero(scf[:, 0:1])
                wnb = wn_sb[:, o:o + WL].unsqueeze(1).to_broadcast([P, C, WL])
                nc.vector._custom_dve(SCAN_MUL, out=scf[:, 1:1 + S2],
                                      in0=msg[:, :S2], in1=wnb)
                R = scf[:, :(C * WB + 1) * L].rearrange("p (x s) -> p x s",
                                                        s=L)
                agg = pool.tile([P, C * WB], f32, tag="agg")
                nc.vector.tensor_tensor(
                    out=agg[:], in0=R[:, 1:1 + C * WB, 0:1].squeeze(-1),
                    in1=R[:, 0:C * WB, 0:1].squeeze(-1),
                    op=mybir.AluOpType.subtract)
                # d = (agg[:,1,:] + db2) - agg[:,0,:]
                d = pool.tile([P, WB], f32, tag="d")
                nc.vector.scalar_tensor_tensor(
                    out=d[:], in0=agg[:, WB:2 * WB], scalar=db2_sb[:, 0:1],
                    in1=agg[:, 0:WB], op0=mybir.AluOpType.add,
                    op1=mybir.AluOpType.subtract)
                # out0 = -softplus(d), out1 = -softplus(-d)
                # (softplus via Ln(Exp(d) + 1): Softplus has no ACT table here)
                st = pool.tile([P, WB, C], f32, tag="st")
                AF = mybir.ActivationFunctionType
                for sgn, oi in ((1.0, 0), (-1.0, 1)):
                    e = pool.tile([P, WB], f32, tag=f"e{oi}")
                    nc.scalar.activation(out=e[:], in_=d[:], func=AF.Exp,
                                         scale=sgn)
                    sp = pool.tile([P, WB], f32, tag=f"sp{oi}")
                    nc.scalar.activation(out=sp[:], in_=e[:], func=AF.Ln,
                                         bias=1.0)
                    nc.scalar.activation(out=st[:, :, oi:oi + 1].squeeze(-1),
                                         in_=sp[:], func=AF.Copy, scale=-1.0)
                nc.scalar.dma_start(out=out_v[:, b * WB:(b + 1) * WB, :],
                                    in_=st[:])
    nc.compile()
    return nc


# ----------------------------------------------------------------------------
# Full model
# ----------------------------------------------------------------------------

_CACHE = {}


def _get_programs(key, Lb, off, T):
    if key not in _CACHE:
        _CACHE[key] = (build_xw_program(),
                       build_aggB_program(Lb, off, T),
                       build_aggC_program(Lb, off, T))
    return _CACHE[key]


def _run(nc, in_maps, trace=False):
    return bass_utils.run_bass_kernel_spmd(
        nc, in_maps, core_ids=list(range(NCORES)),
        trace=trace, trace_cores=[0] if trace else None)


def kernel(x, edge_index, edge_weight, W1, b1, W2, b2, _timing=False):
    register_custom_ops()
    x = np.asarray(x, np.float32)
    W1 = np.asarray(W1, np.float32); b1 = np.asarray(b1, np.float32)
    W2 = np.asarray(W2, np.float32); b2 = np.asarray(b2, np.float32)

    g = preprocess_graph(edge_index, edge_weight)
    Lb, off, T = g["Lb"], g["off"], g["T"]
    progA, progB, progC = _get_programs(tuple(Lb), Lb, off, T)
    times = {}

    # --- kernel A: h = x @ W1 ---
    xT = x.T.astype(np.float16)                       # [37, N]
    W1h = W1.astype(np.float16)
    inA = []
    for c in range(NCORES):
        xp = np.zeros((F_IN, NPAD), np.float16)
        xp[:, :DPC] = xT[:, c * DPC:(c + 1) * DPC]
        inA.append({"xT": xp, "W1": W1h})
    rA = _run(progA, inA, trace=_timing)
    times["A"] = rA.exec_time_ns
    hfull = np.concatenate([rA.results[c]["hT"][:, :DPC]
                            for c in range(NCORES)], axis=1)  # [16, N] f16

    # --- kernel B: aggregate h, relu, zw = z @ W2 ---
    b1r = np.broadcast_to(b1, (P, H)).astype(np.float32).copy()
    W2f = W2.astype(np.float32)
    inB = []
    for c in range(NCORES):
        gat = hfull[:, g["srcpos"][c]].transpose(1, 0, 2)  # [P, 16, T] f16
        parts = [gat[:, :, off[b]:off[b + 1]].reshape(P, -1)
                 for b in range(NB)]
        msg = np.ascontiguousarray(np.concatenate(parts, 1))
        inB.append({"msg": msg, "wn": g["wn"][c], "b1": b1r, "W2": W2f})
    rB = _run(progB, inB, trace=_timing)
    times["B"] = rB.exec_time_ns

    # --- kernel C: aggregate zw, + b2, log_softmax ---
    zwfull = np.empty((C, N), np.float32)
    for c in range(NCORES):
        order, _ = g["perms"][c]
        zwfull[:, c * DPC + order] = rB.results[c]["zw"][:, :DPC]
    zwh = zwfull.astype(np.float16)
    db2 = np.full((P, 1), float(b2[1] - b2[0]), np.float32)
    inC = []
    for c in range(NCORES):
        gat = zwh[:, g["srcpos"][c]].transpose(1, 0, 2)    # [P, 2, T] f16
        parts = [gat[:, :, off[b]:off[b + 1]].reshape(P, -1)
                 for b in range(NB)]
        msg = np.ascontiguousarray(np.concatenate(parts, 1))
        inC.append({"msg": msg, "wn": g["wn"][c], "db2": db2})
    rC = _run(progC, inC, trace=_timing)
    times["C"] = rC.exec_time_ns

    out = np.empty((N, C), np.float32)
    for c in range(NCORES):
        order, _ = g["perms"][c]
        out[c * DPC + order] = rC.results[c]["out"][:DPC]
    if _timing:
        kernel._last_runs = {"A": rA, "B": rB, "C": rC}
        return out, times
    return out
